# revision 17
# baseline (speedup 1.0000x reference)
"""TRN2 Bass kernel for nn_CompiledBlock_45148696216108 (moe_routing).

Reference computation:
    xp  = x[permute]
    xn  = LayerNorm(xp; gamma, beta, eps=1e-5)
    yp  = xn @ W.T + b
    out = (xp + yp)[argsort(permute)]

The block (LayerNorm + Linear + residual) is purely row-wise, so the
gather by `permute` and the scatter by its inverse cancel exactly:
    out = x + LN(x) @ W.T + b
No token movement (and no cross-core all-to-all) is needed. Tokens are
sharded contiguously across the 8 cores; the tiny weights are folded on
the host and replicated:
    A[h, o] = gamma[h] * W[o, h]     (pre-transposed, gamma folded, bf16)
    S[o]    = sum_h A[h, o]          (column sums of the bf16 A)
    c[o]    = sum_h beta[h] * W[o, h] + b[o]
and LayerNorm's affine is applied AFTER the matmul (exact rewrite):
    out = (x + c + (-mu*rsig) * S) + rsig * (x @ A)
x is consumed entirely as a host-cast bf16 copy (matmul, stats, and
residual); the bf16 rounding is ~0.4% against a 2e-2 budget.

Per-core device pipeline (tokens_per_core = 8192, HIDDEN = 2048):
  - xb streams twice from HBM: natural layout (SP queue) for
    stats/residual, and through the XBAR DMA-transpose (ACT queue)
    directly into [h%128, h//128, t] layout. The PE's input chain is
    just boot -> xbT tile -> A chunk: no stats/normalize/transpose
    latency in front of the matmuls, and the PE runs nothing but bf16
    matmuls (LDWEIGHTS 97ns, hidden under the 512-row streams).
  - DVE bn_stats/bn_aggr -> mean/var; ACT sqrt + DVE reciprocal -> rsig,
    nmr = -mu*rsig (only the combines consume these)
  - ACT: v = nmr*S;  GPSIMD: xc = xb + c, then xc += v  (both off-path;
    xc is a fresh tile so the xb stream never waits on GPSIMD)
  - PE: psum_j[t, o512] = sum_k xbT[k].T @ A[k, j]  (j-outer; tiles 0/1
    k-outer so matmuls consume each A chunk as its DMA lands)
  - DVE per-bank: out_j = rsig * psum_j + xc_j  (one pass); DMA out on
    the SP queue; the last two tiles DMA per-bank to shorten the tail.
Deep buffer rings (xb x5, xbT x4, v/xc/out x3, psum x2) keep every
cross-engine WAR release at least a tile ahead of its consumer.
"""

import numpy as np
from contextlib import ExitStack

from concourse import bacc, tile, mybir
from concourse.bass_utils import run_bass_kernel_spmd
from concourse.masks import make_identity

N_TOK = 65536
HIDDEN = 2048
N_CORES = 8
P = 128
EPS = 1e-5
F32 = mybir.dt.float32
BF16 = mybir.dt.bfloat16
AF = mybir.ActivationFunctionType
ALU = mybir.AluOpType

NB = 512               # matmul free dim (one PSUM bank of fp32)
KC = HIDDEN // P       # 16 contraction chunks
OC = HIDDEN // NB      # 4 output column chunks


def build(tokens_per_core: int = N_TOK // N_CORES, num_devices: int = N_CORES):
    T = tokens_per_core
    NT = T // P            # token tiles

    nc = bacc.Bacc(
        "TRN2", target_bir_lowering=False, debug=False, num_devices=num_devices
    )
    xb_d = nc.dram_tensor("xb", [T, HIDDEN], BF16, kind="ExternalInput").ap()
    a_d = nc.dram_tensor("A", [KC, P, HIDDEN], BF16, kind="ExternalInput").ap()
    c_d = nc.dram_tensor("c", [P, HIDDEN], BF16, kind="ExternalInput").ap()
    s_d = nc.dram_tensor("S", [P, HIDDEN], BF16, kind="ExternalInput").ap()
    out_d = nc.dram_tensor("out", [T, HIDDEN], F32, kind="ExternalOutput").ap()

    with tile.TileContext(nc) as tc, ExitStack() as ctx:
        const = ctx.enter_context(tc.tile_pool(name="const", bufs=1))
        apool = ctx.enter_context(tc.tile_pool(name="apool", bufs=1))
        xpool = ctx.enter_context(tc.tile_pool(name="xpool", bufs=5))
        xtpool = ctx.enter_context(tc.tile_pool(name="xtpool", bufs=4))
        vpool = ctx.enter_context(tc.tile_pool(name="vpool", bufs=3))
        xcpool = ctx.enter_context(tc.tile_pool(name="xcpool", bufs=3))
        outpool = ctx.enter_context(tc.tile_pool(name="outpool", bufs=3))
        stpool = ctx.enter_context(tc.tile_pool(name="stats", bufs=3))
        psy_pool = ctx.enter_context(tc.tile_pool(name="psy", bufs=2, space="PSUM"))

        def load_xb(i):
            xb = xpool.tile([P, HIDDEN], BF16, tag="xb", name=f"xb_{i}")
            nc.sync.dma_start(xb[:], xb_d[i * P : (i + 1) * P, :])
            return xb

        def load_xbt(i):
            xbt = xtpool.tile([P, KC, P], BF16, tag="xbt", name=f"xbt_{i}")
            nc.scalar.dma_start_transpose(xbt[:], xb_d[i * P : (i + 1) * P, :])
            return xbt

        # PE-feeding streams first on each queue: the transposed bf16 tiles
        # and odd A chunks on the ACT queue; natural-layout tiles, c/S and
        # even A chunks on the SP queue (A0/A2 early: tile 0 runs k-outer).
        xbs, xbts, sts = {}, {}, {}
        a_sb = [None] * KC

        def load_a(k, eng):
            a_sb[k] = apool.tile([P, HIDDEN], BF16, tag=f"a{k}", name=f"a_{k}")
            eng.dma_start(a_sb[k][:], a_d[k])

        # A chunks lead both queues (tile 0/1 consume them k-outer as they
        # land); xb1/xb2/c/S trail on the ACT queue, needed only by the
        # stats/residual path which first matters at combine(0) ~36us.
        xbts[0] = load_xbt(0)          # ACT queue head: PE's first input
        load_a(1, nc.scalar)
        load_a(0, nc.sync)
        load_a(2, nc.sync)
        load_a(3, nc.scalar)
        xbts[1] = load_xbt(1)
        xbs[0] = load_xb(0)
        for k in range(4, KC):
            load_a(k, nc.sync if k % 2 == 0 else nc.scalar)
        xbs[1] = load_xb(1)
        c_sb = const.tile([P, HIDDEN], BF16)
        nc.scalar.dma_start(c_sb[:], c_d[:])
        s_sb = const.tile([P, HIDDEN], BF16)
        nc.scalar.dma_start(s_sb[:], s_d[:])
        xbs[2] = load_xb(2)
        eps_sb = const.tile([P, 1], F32)
        nc.gpsimd.memset(eps_sb[:], EPS)

        # HAM pre-warm: back-to-back identity transposes keep the PE busy
        # through the boot window, so its clock gate is already at 8/8
        # (2.4 GHz) when the first real matmul issues at ~17us. They
        # borrow one psys buffer (tag psy0); tile 1 reuses it much later.
        ident = const.tile([P, P], F32)
        make_identity(nc, ident[:])
        wp = psy_pool.tile([P, NB], F32, tag="psy0", name="warm_psum")
        for w in range(36):
            nc.tensor.transpose(wp[:, 0:P], ident[:], ident[:])

        def stats(i, xb):
            """LN stats for tile i -> (rsig, nmr); GPSIMD folds xb+c -> xc.

            xc is a fresh tile (not in-place) so the next xb DMA into this
            buffer only waits on reads that finish early in the pipeline.
            """
            st = stpool.tile([P, 4, 6], F32, tag="stats")
            xr = xb[:].rearrange("p (a b) -> p a b", b=512)
            for a in range(4):
                nc.vector.bn_stats(st[:, a, :], xr[:, a, :])
            mv = stpool.tile([P, 2], F32, tag="mv")
            nc.vector.bn_aggr(mv[:], st[:])
            sig = stpool.tile([P, 1], F32, tag="sig")
            nc.scalar.activation(sig[:], mv[:, 1:2], AF.Sqrt, bias=eps_sb[:])
            rsig = stpool.tile([P, 1], F32, tag="rsig")
            nc.vector.reciprocal(rsig[:], sig[:])
            nmr = stpool.tile([P, 1], F32, tag="nmr")
            nc.vector.scalar_tensor_tensor(
                nmr[:], mv[:, 0:1], -1.0, rsig[:], ALU.mult, ALU.mult
            )
            xc = xcpool.tile([P, HIDDEN], F32, tag="xc", name=f"xc_{i}")
            nc.gpsimd.tensor_add(xc[:], xb[:], c_sb[:])
            return rsig, nmr, xc

        sts[0] = stats(0, xbs[0])

        for t in range(NT):
            xbs.pop(t)
            xbt = xbts.pop(t)
            rsig, nmr, xc = sts.pop(t)

            # Fold the mean-correction term into the residual now that
            # nmr(t) exists: v = nmr*S on the otherwise-idle ACT engine,
            # then xc += v on GPSIMD, both well before the combines need xc.
            v = vpool.tile([P, HIDDEN], F32, tag="v", name=f"v_{t}")
            nc.scalar.activation(v[:], s_sb[:], AF.Copy, scale=nmr[:])
            nc.gpsimd.tensor_add(xc[:], xc[:], v[:])

            if t + 1 < NT:
                sts[t + 1] = stats(t + 1, xbs[t + 1])
            if t + 2 < NT:
                xbts[t + 2] = load_xbt(t + 2)
            if t + 3 < NT:
                xbs[t + 3] = load_xb(t + 3)

            # Matmuls, one PSUM bank (512 outputs) at a time; combine per
            # bank folds the LayerNorm scale: out = rsig*psum + xc.
            ot = outpool.tile([P, HIDDEN], F32, tag="ot")
            psys = [psy_pool.tile([P, NB], F32, tag=f"psy{j}", name=f"psy_{t}_{j}")
                    for j in range(OC)]
            order = (
                [(j, k) for k in range(KC) for j in range(OC)]
                if t <= 3
                else [(j, k) for j in range(OC) for k in range(KC)]
            )
            for j, k in order:
                nc.tensor.matmul(
                    psys[j][:],
                    xbt[:, k, :],
                    a_sb[k][:, j * NB : (j + 1) * NB],
                    start=(k == 0),
                    stop=(k == KC - 1),
                )
                if k == KC - 1:
                    sl = slice(j * NB, (j + 1) * NB)
                    nc.vector.scalar_tensor_tensor(
                        ot[:, sl], psys[j][:], rsig[:], xc[:, sl],
                        ALU.mult, ALU.add,
                    )
                    if t >= NT - 2:
                        # tail tiles: per-bank DMA right after each combine
                        nc.sync.dma_start(
                            out_d[t * P : (t + 1) * P, sl], ot[:, sl]
                        )

            if t < NT - 2:
                nc.sync.dma_start(out_d[t * P : (t + 1) * P, :], ot[:])

    nc.compile()
    return nc


_built = None


def _get_built():
    global _built
    if _built is None:
        _built = build()
    return _built


def _prep_inputs(x, permute, gamma, beta, W, b):
    import ml_dtypes

    x = np.asarray(x, dtype=np.float32)
    gamma = np.asarray(gamma, dtype=np.float32)
    beta = np.asarray(beta, dtype=np.float32)
    W = np.asarray(W, dtype=np.float32)
    b = np.asarray(b, dtype=np.float32)
    A = np.ascontiguousarray(W.T) * gamma[:, None]  # (H, O), gamma folded
    A = A.astype(ml_dtypes.bfloat16)
    S = np.ascontiguousarray(
        np.broadcast_to(
            A.astype(np.float64).sum(0, keepdims=True).astype(np.float32),
            (P, HIDDEN),
        )
    ).astype(ml_dtypes.bfloat16)
    A_in = np.ascontiguousarray(A.reshape(HIDDEN // P, P, HIDDEN))
    c = np.ascontiguousarray(
        np.broadcast_to((W @ beta + b).reshape(1, HIDDEN), (P, HIDDEN))
    ).astype(ml_dtypes.bfloat16)
    xb = x.astype(ml_dtypes.bfloat16)
    T = N_TOK // N_CORES
    in_maps = []
    for i in range(N_CORES):
        sl = slice(i * T, (i + 1) * T)
        in_maps.append({"xb": xb[sl], "A": A_in, "c": c, "S": S})
    return in_maps


def kernel(x, permute, gamma, beta, W, b):
    nc = _get_built()
    in_maps = _prep_inputs(x, permute, gamma, beta, W, b)
    res = run_bass_kernel_spmd(nc, in_maps, list(range(N_CORES))).results
    return np.concatenate([r["out"] for r in res], axis=0)


if __name__ == "__main__":
    rng = np.random.default_rng(0)
    x = rng.standard_normal((N_TOK, HIDDEN), dtype=np.float32)
    permute = rng.permutation(N_TOK).astype(np.int64)
    gamma = np.ones(HIDDEN, np.float32)
    beta = np.zeros(HIDDEN, np.float32)
    W = (rng.standard_normal((HIDDEN, HIDDEN), dtype=np.float32) / np.sqrt(HIDDEN))
    b = rng.standard_normal(HIDDEN, dtype=np.float32) * 0.01
    out = kernel(x=x, permute=permute, gamma=gamma, beta=beta, W=W, b=b)
    print(out.shape, out.dtype)
